# revision 24
# baseline (speedup 1.0000x reference)
"""CoordAtt (coordinate attention) Trainium2 Bass kernel.

Data-parallel over 8 NeuronCores: one batch sample per core.
Per-core layout: channels on partitions (2 blocks of 128), [H*W] on free dim.

Math (per sample):
  xh[c,h] = sum_w x[c,h,w];  xw[c,w] = sum_h x[c,h,w]          (pool, DVE)
  d_h = relu(W1eff @ xh + b1eff)   (BN + 1/W + conv folded on host, PE)
  d_w = relu(W2eff @ xw + b2eff)
  ah  = sigmoid(us_w @ d_h + us_b) ; aw = sigmoid(us_w @ d_w + us_b)
  o   = x*(1 + ah*aw);  out = hswish(o) = o*clip(o/6+1/2, 0, 1)

h-swish is computed exactly via:
  ah' = -ah/6 ; s' = ah'*aw ; o' = (s' - 1/6)*x          ( = -o/6 )
  r1 = relu(-o' + 1/2) ; r2 = relu(-6*r1 + 6) ; out = (r2 - 6)*o'
"""

import sys

if "/opt/trn_rl_repo" not in sys.path:
    sys.path.insert(0, "/opt/trn_rl_repo")

import numpy as np

import concourse.bass as bass
import concourse.mybir as mybir
from concourse.bass_utils import run_bass_kernel_spmd
from concourse.tile import TileContext

F32 = mybir.dt.float32
AF = mybir.ActivationFunctionType
OP = mybir.AluOpType
AX = mybir.AxisListType

N, C, H, W = 8, 256, 112, 112
HW = H * W
NBLK = 2  # channel blocks of 128
P = 128
MIP = 8
EPS = 1e-5

# chunk sizes (rows of h)
LOAD_HC = 28   # 4 load chunks per block
APPLY_HC = 14  # 8 apply chunks per block

_CACHE: dict = {}
PPK = 294  # packed param tile width
import os as _os

USE_FUSED = _os.environ.get("COORDATT_FUSED", "1") == "1"


def _hswish_op():
    """Register (once) a custom DVE op computing the whole apply tail:
    out = o * clip(o*s1 + imm2, 0, s0), o = (in0 + s0) * in1
    with s0=1, s1=1/6, imm2=1/2 this is hswish(x*(1+ah*aw)) given
    in0 = ah*aw, in1 = x.
    """
    from concourse import dve_ops as _dvo
    from concourse.dve_spec import Spec, Src0, Src1, C0, C1, C2, relu, minn

    name = "HSWISH_APPLY_ANT"
    for op in _dvo.OPS:
        if op.name == name:
            return op
    o = (Src0 + C0) * Src1

    def _ref(in0, in1, s0, s1, imm2):
        oo = ((in0.astype(np.float32) + s0) * in1).astype(np.float32)
        return (oo * np.clip(oo * s1 + imm2, 0.0, s0)).astype(np.float32)

    spec = Spec(body=o * relu(minn(o * C1 + C2, C0)), reference=_ref)
    op = _dvo.DveOp(
        name, spec, subdim=False,
        uops_sha={"v3": "4af51ed3c9ed553f", "v4": "b44642dd784bda88"},
    )
    _dvo.OPS.append(op)
    _dvo.CUSTOM_DVE_SPECS[name] = spec
    _dvo._SUB_OPCODE_FOR_NAME[name] = (
        _dvo._CUSTOM_DVE_ROW_BASE + len(_dvo.OPS) - 1
    )
    return op


def _build_program() -> bass.Bass:
    from concourse.bacc import Bacc

    nc = Bacc()

    x = nc.declare_dram_parameter("x", [C, H, W], F32, isOutput=False)
    pp = nc.declare_dram_parameter("pp", [P, PPK], F32, isOutput=False)
    out = nc.declare_dram_parameter("out", [C, H, W], F32, isOutput=True)

    n_load = H // LOAD_HC
    n_apply = H // APPLY_HC

    with TileContext(nc) as tc:
        with (
            tc.tile_pool(name="xres", bufs=1) as xpool,
            tc.tile_pool(name="small", bufs=1) as spool,
            tc.tile_pool(name="work", bufs=3) as wpool,
            tc.tile_pool(name="psum", bufs=2, space="PSUM") as ppool,
        ):
            # ---- resident x tiles + param tiles ----
            xt = [xpool.tile([P, HW], F32, tag=f"x{b}", name=f"x{b}") for b in range(NBLK)]
            pps = spool.tile([P, PPK], F32, tag="pps")
            ppt = spool.tile([P, PPK], F32, tag="pp")
            # packed param views (see _pack_params)
            w1t = ppt[:, 0:16]
            w2t = ppt[:, 16:32]
            uswt = ppt[0:MIP, 32:32 + C]
            usbt = ppt[:, 288:290]
            bbt = ppt[0:MIP, 290:292]
            cstt = ppt[:, 292:294]
            xh = [spool.tile([P, H], F32, tag=f"xh{b}", name=f"xh{b}") for b in range(NBLK)]
            xw4 = [spool.tile([P, 4 * W], F32, tag=f"xw4{b}", name=f"xw4{b}") for b in range(NBLK)]
            xw = [spool.tile([P, W], F32, tag=f"xw{b}", name=f"xw{b}") for b in range(NBLK)]
            dh = spool.tile([MIP, H], F32, tag="dh")
            dw = spool.tile([MIP, W], F32, tag="dw")
            ahp = [spool.tile([P, H], F32, tag=f"ahp{b}", name=f"ahp{b}") for b in range(NBLK)]
            aw = [spool.tile([P, W], F32, tag=f"aw{b}", name=f"aw{b}") for b in range(NBLK)]

            # single packed param load; bounce through DVE so downstream
            # matmuls depend on one DVE sem instead of a DMA-queue sem
            # (the Matmult LW struct carries very few sync waits).
            nc.sync.dma_start(out=pps[:, :], in_=pp[:, :])
            nc.vector.tensor_copy(ppt[:, :], pps[:, :])
            # ACT warmup: make the scalar engine observe the DVE sem now so
            # the first sigmoid later needs only the PE-sem wait (ISA allows
            # very few sync waits per instruction).
            warm = spool.tile([P, 1], F32, tag="warm")
            nc.scalar.copy(warm[:, :], cstt[:, 0:1])

            # ---- phase 1: load x, pooled sums ----
            for b in range(NBLK):
                for j in range(n_load):
                    h0 = j * LOAD_HC
                    sl = slice(h0 * W, (h0 + LOAD_HC) * W)
                    nc.sync.dma_start(
                        out=xt[b][:, sl],
                        in_=x[b * P:(b + 1) * P, h0:h0 + LOAD_HC, :],
                    )
                    xv = xt[b][:, sl].rearrange("p (h w) -> p h w", h=LOAD_HC)
                    # xh chunk: contiguous reduce over w
                    nc.vector.tensor_reduce(
                        xh[b][:, h0:h0 + LOAD_HC], xv, AX.X, OP.add
                    )
                    # xw partial: strided reduce over h
                    xvt = xt[b][:, sl].rearrange("p (h w) -> p w h", h=LOAD_HC)
                    nc.vector.tensor_reduce(
                        xw4[b][:, j * W:(j + 1) * W], xvt, AX.X, OP.add
                    )
                # fold the 4 partials
                nc.vector.tensor_reduce(
                    xw[b][:, :],
                    xw4[b][:, :].rearrange("p (j w) -> p w j", j=4),
                    AX.X,
                    OP.add,
                )

            # ---- conv chain (tiny) ----
            ph = ppool.tile([MIP, H], F32, tag="ph")
            pw = ppool.tile([MIP, W], F32, tag="pw")
            for b in range(NBLK):
                nc.tensor.matmul(
                    ph[:, :], w1t[:, b * MIP:(b + 1) * MIP], xh[b][:, :],
                    start=(b == 0), stop=(b == NBLK - 1),
                )
            for b in range(NBLK):
                nc.tensor.matmul(
                    pw[:, :], w2t[:, b * MIP:(b + 1) * MIP], xw[b][:, :],
                    start=(b == 0), stop=(b == NBLK - 1),
                )
            # relu(+bias) on DVE (keeps ACT tables on Sigmoid/Relu only)
            nc.vector.tensor_scalar(
                dh[:, :], ph[:, :], bbt[:, 0:1], 0.0, OP.add, OP.max
            )
            nc.vector.tensor_scalar(
                dw[:, :], pw[:, :], bbt[:, 1:2], 0.0, OP.add, OP.max
            )

            for b in range(NBLK):
                p2h = ppool.tile([P, H], F32, tag="p2h")
                p2w = ppool.tile([P, W], F32, tag="p2w")
                nc.tensor.matmul(
                    p2h[:, :], uswt[:, b * P:(b + 1) * P], dh[:, :],
                    start=True, stop=True,
                )
                nc.tensor.matmul(
                    p2w[:, :], uswt[:, b * P:(b + 1) * P], dw[:, :],
                    start=True, stop=True,
                )
                # ah' = -sigmoid(p2h + usb)/6 : sigmoid first, then scale
                nc.scalar.activation(
                    ahp[b][:, :], p2h[:, :], AF.Sigmoid, bias=usbt[:, b:b + 1]
                )
                nc.scalar.activation(
                    aw[b][:, :], p2w[:, :], AF.Sigmoid, bias=usbt[:, b:b + 1]
                )
            if not USE_FUSED:
                for b in range(NBLK):
                    nc.vector.tensor_scalar(
                        ahp[b][:, :], ahp[b][:, :], -1.0 / 6.0, None, OP.mult
                    )
            hsw = _hswish_op() if USE_FUSED else None

            # ---- phase 2: apply + store ----
            for b in range(NBLK):
                for j in range(n_apply):
                    h0 = j * APPLY_HC
                    sl = slice(h0 * W, (h0 + APPLY_HC) * W)
                    fd = APPLY_HC * W
                    sp = wpool.tile([P, fd], F32, tag="s")
                    f = wpool.tile([P, fd], F32, tag="f")

                    ah_v = (
                        ahp[b][:, h0:h0 + APPLY_HC]
                        .unsqueeze(2)
                        .broadcast_to([P, APPLY_HC, W])
                    )
                    aw_v = (
                        aw[b][:, :].unsqueeze(1).broadcast_to([P, APPLY_HC, W])
                    )
                    s3 = sp[:, :].rearrange("p (h w) -> p h w", h=APPLY_HC)
                    nc.vector.tensor_tensor(s3, ah_v, aw_v, OP.mult)
                    if USE_FUSED:
                        nc.vector._custom_dve(
                            hsw, out=f[:, :], in0=sp[:, :], in1=xt[b][:, sl],
                            s0=1.0, s1=1.0 / 6.0, imm2=0.5,
                        )
                    else:
                        op_ = wpool.tile([P, fd], F32, tag="o")
                        r1 = wpool.tile([P, fd], F32, tag="r1")
                        r2 = wpool.tile([P, fd], F32, tag="r2")
                        nc.vector.scalar_tensor_tensor(
                            op_[:, :], sp[:, :], -1.0 / 6.0, xt[b][:, sl],
                            OP.add, OP.mult,
                        )
                        nc.scalar.activation(
                            r1[:, :], op_[:, :], AF.Relu,
                            bias=cstt[:, 0:1], scale=-1.0,
                        )
                        nc.scalar.activation(
                            r2[:, :], r1[:, :], AF.Relu,
                            bias=cstt[:, 1:2], scale=-6.0,
                        )
                        nc.vector.scalar_tensor_tensor(
                            f[:, :], r2[:, :], -6.0, op_[:, :], OP.add, OP.mult
                        )
                    nc.sync.dma_start(
                        out=out[b * P:(b + 1) * P, h0:h0 + APPLY_HC, :],
                        in_=f[:, :].rearrange("p (h w) -> p h w", h=APPLY_HC),
                    )
    nc.compile()
    return nc


def _fold_params(bn1_gamma, bn1_beta, bn1_mean, bn1_var,
                 bn2_gamma, bn2_beta, bn2_mean, bn2_var,
                 ds_w, ds_b, us_w, us_b):
    s1 = bn1_gamma / np.sqrt(bn1_var + EPS)
    t1 = bn1_beta - bn1_mean * s1
    s2 = bn2_gamma / np.sqrt(bn2_var + EPS)
    t2 = bn2_beta - bn2_mean * s2
    # d = relu(ds_w @ (sum/W * s + t) + ds_b) = relu(W_eff @ sum + b_eff)
    w1e = (ds_w * s1[None, :] / W).astype(np.float32)      # [MIP, C]
    w2e = (ds_w * s2[None, :] / H).astype(np.float32)
    b1e = (ds_w @ t1 + ds_b).astype(np.float32)            # [MIP]
    b2e = (ds_w @ t2 + ds_b).astype(np.float32)
    # lhsT layouts
    w1_l = w1e.T.reshape(NBLK, P, MIP)                     # [blk, c, o]
    w2_l = w2e.T.reshape(NBLK, P, MIP)
    usw_l = us_w.T.astype(np.float32)                      # [MIP, C]
    usb_l = us_b.reshape(NBLK, P)
    bb_l = np.stack([b1e, b2e], axis=1)                    # [MIP, 2]

    pp = np.zeros((P, PPK), np.float32)
    pp[:, 0:MIP] = w1_l[0]
    pp[:, MIP:2 * MIP] = w1_l[1]
    pp[:, 16:16 + MIP] = w2_l[0]
    pp[:, 16 + MIP:32] = w2_l[1]
    pp[0:MIP, 32:32 + C] = usw_l
    pp[:, 288] = usb_l[0]
    pp[:, 289] = usb_l[1]
    pp[0:MIP, 290:292] = bb_l
    pp[:, 292] = 0.5
    pp[:, 293] = 6.0
    return np.ascontiguousarray(pp)


def kernel(**inputs) -> np.ndarray:
    x = np.asarray(inputs["x"], dtype=np.float32)
    pp = _fold_params(
        np.asarray(inputs["bn1_gamma"], np.float32),
        np.asarray(inputs["bn1_beta"], np.float32),
        np.asarray(inputs["bn1_mean"], np.float32),
        np.asarray(inputs["bn1_var"], np.float32),
        np.asarray(inputs["bn2_gamma"], np.float32),
        np.asarray(inputs["bn2_beta"], np.float32),
        np.asarray(inputs["bn2_mean"], np.float32),
        np.asarray(inputs["bn2_var"], np.float32),
        np.asarray(inputs["ds_w"], np.float32),
        np.asarray(inputs["ds_b"], np.float32),
        np.asarray(inputs["us_w"], np.float32),
        np.asarray(inputs["us_b"], np.float32),
    )
    if "nc" not in _CACHE:
        _CACHE["nc"] = _build_program()
    nc = _CACHE["nc"]

    in_maps = [
        {"x": np.ascontiguousarray(x[i]), "pp": pp} for i in range(N)
    ]
    res = run_bass_kernel_spmd(nc, in_maps, core_ids=list(range(N)))
    return np.stack([r["out"] for r in res.results]).astype(np.float32)


if __name__ == "__main__":
    rng = np.random.default_rng(0)
    demo = {
        "x": rng.standard_normal((N, C, H, W), dtype=np.float32),
        "bn1_gamma": rng.random(C, dtype=np.float32),
        "bn1_beta": rng.standard_normal(C).astype(np.float32) * 0.1,
        "bn1_mean": rng.standard_normal(C).astype(np.float32) * 0.1,
        "bn1_var": rng.random(C, dtype=np.float32) + 0.5,
        "bn2_gamma": rng.random(C, dtype=np.float32),
        "bn2_beta": rng.standard_normal(C).astype(np.float32) * 0.1,
        "bn2_mean": rng.standard_normal(C).astype(np.float32) * 0.1,
        "bn2_var": rng.random(C, dtype=np.float32) + 0.5,
        "ds_w": (rng.standard_normal((MIP, C)) / np.sqrt(C)).astype(np.float32),
        "ds_b": (rng.standard_normal(MIP) * 0.01).astype(np.float32),
        "us_w": (rng.standard_normal((C, MIP)) / np.sqrt(MIP)).astype(np.float32),
        "us_b": (rng.standard_normal(C) * 0.01).astype(np.float32),
    }
    o = kernel(**demo)
    print("kernel output", o.shape, o.dtype)


# revision 27
# speedup vs baseline: 1.1248x; 1.1248x over previous
"""CoordAtt (coordinate attention) Trainium2 Bass kernel.

Data-parallel over 8 NeuronCores: one batch sample per core.
Per-core layout: channels on partitions (2 blocks of 128), [H*W] on free dim.

Math (per sample):
  xh[c,h] = sum_w x[c,h,w];  xw[c,w] = sum_h x[c,h,w]          (pool, DVE)
  d_h = relu(W1eff @ xh + b1eff)   (BN + 1/W + conv folded on host, PE)
  d_w = relu(W2eff @ xw + b2eff)
  ah  = sigmoid(us_w @ d_h + us_b) ; aw = sigmoid(us_w @ d_w + us_b)
  o   = x*(1 + ah*aw);  out = hswish(o) = o*clip(o/6+1/2, 0, 1)

h-swish is computed exactly via:
  ah' = -ah/6 ; s' = ah'*aw ; o' = (s' - 1/6)*x          ( = -o/6 )
  r1 = relu(-o' + 1/2) ; r2 = relu(-6*r1 + 6) ; out = (r2 - 6)*o'
"""

import sys

if "/opt/trn_rl_repo" not in sys.path:
    sys.path.insert(0, "/opt/trn_rl_repo")

import numpy as np

import concourse.bass as bass
import concourse.mybir as mybir
from concourse.bass_utils import run_bass_kernel_spmd
from concourse.tile import TileContext

F32 = mybir.dt.float32
AF = mybir.ActivationFunctionType
OP = mybir.AluOpType
AX = mybir.AxisListType

N, C, H, W = 8, 256, 112, 112
HW = H * W
NBLK = 2  # channel blocks of 128
P = 128
MIP = 8
EPS = 1e-5

# chunk sizes (rows of h)
LOAD_HC = 28   # 4 load chunks per block
APPLY_HC = 14  # 8 apply chunks per block

_CACHE: dict = {}
PPK = 294  # packed param tile width
import os as _os

USE_FUSED = _os.environ.get("COORDATT_FUSED", "1") == "1"


def _hswish_op():
    """Register (once) a custom DVE op computing the whole apply tail:
    out = o * clip(o*s1 + imm2, 0, s0), o = (in0 + s0) * in1
    with s0=1, s1=1/6, imm2=1/2 this is hswish(x*(1+ah*aw)) given
    in0 = ah*aw, in1 = x.
    """
    from concourse import dve_ops as _dvo
    from concourse.dve_spec import Spec, Src0, Src1, C0, C1, C2, relu, minn

    name = "HSWISH_APPLY_ANT"
    for op in _dvo.OPS:
        if op.name == name:
            return op
    o = (Src0 + C0) * Src1

    def _ref(in0, in1, s0, s1, imm2):
        oo = ((in0.astype(np.float32) + s0) * in1).astype(np.float32)
        return (oo * np.clip(oo * s1 + imm2, 0.0, s0)).astype(np.float32)

    spec = Spec(body=o * relu(minn(o * C1 + C2, C0)), reference=_ref)
    op = _dvo.DveOp(
        name, spec, subdim=False,
        uops_sha={"v3": "4af51ed3c9ed553f", "v4": "b44642dd784bda88"},
    )
    _dvo.OPS.append(op)
    _dvo.CUSTOM_DVE_SPECS[name] = spec
    _dvo._SUB_OPCODE_FOR_NAME[name] = (
        _dvo._CUSTOM_DVE_ROW_BASE + len(_dvo.OPS) - 1
    )
    return op


def _build_program() -> bass.Bass:
    from concourse.bacc import Bacc

    nc = Bacc()

    x = nc.declare_dram_parameter("x", [C, H, W], F32, isOutput=False)
    pp = nc.declare_dram_parameter("pp", [P, PPK], F32, isOutput=False)
    out = nc.declare_dram_parameter("out", [C, H, W], F32, isOutput=True)

    n_load = H // LOAD_HC
    n_apply = H // APPLY_HC

    with TileContext(nc) as tc:
        with (
            tc.tile_pool(name="xres", bufs=1) as xpool,
            tc.tile_pool(name="small", bufs=1) as spool,
            tc.tile_pool(name="work", bufs=3) as wpool,
            tc.tile_pool(name="psum", bufs=2, space="PSUM") as ppool,
        ):
            # ---- resident x tiles + param tiles ----
            xt = [xpool.tile([P, HW], F32, tag=f"x{b}", name=f"x{b}") for b in range(NBLK)]
            pps = spool.tile([P, PPK], F32, tag="pps")
            ppt = spool.tile([P, PPK], F32, tag="pp")
            # packed param views (see _pack_params)
            w1t = ppt[:, 0:16]
            w2t = ppt[:, 16:32]
            uswt = ppt[0:MIP, 32:32 + C]
            usbt = ppt[:, 288:290]
            bbt = ppt[0:MIP, 290:292]
            cstt = ppt[:, 292:294]
            xh = [spool.tile([P, H], F32, tag=f"xh{b}", name=f"xh{b}") for b in range(NBLK)]
            xw4 = [spool.tile([P, 4 * W], F32, tag=f"xw4{b}", name=f"xw4{b}") for b in range(NBLK)]
            xw = [spool.tile([P, W], F32, tag=f"xw{b}", name=f"xw{b}") for b in range(NBLK)]
            dh = spool.tile([MIP, H], F32, tag="dh")
            dw = spool.tile([MIP, W], F32, tag="dw")
            ahp = [spool.tile([P, H], F32, tag=f"ahp{b}", name=f"ahp{b}") for b in range(NBLK)]
            aw = [spool.tile([P, W], F32, tag=f"aw{b}", name=f"aw{b}") for b in range(NBLK)]

            # single packed param load; bounce through DVE so downstream
            # matmuls depend on one DVE sem instead of a DMA-queue sem
            # (the Matmult LW struct carries very few sync waits).
            nc.sync.dma_start(out=pps[:, :], in_=pp[:, :])
            nc.vector.tensor_copy(ppt[:, :], pps[:, :])
            # ACT warmup: make the scalar engine observe the DVE sem now so
            # the first sigmoid later needs only the PE-sem wait (ISA allows
            # very few sync waits per instruction). The dummy sigmoid also
            # preloads the ACT table outside the critical path.
            warm = spool.tile([P, 1], F32, tag="warm")
            nc.scalar.copy(warm[:, :], cstt[:, 0:1])
            nc.scalar.activation(warm[:, :], cstt[:, 0:1], AF.Sigmoid)

            # ---- phase 1: load x, pooled sums ----
            # Per 28-row chunk: GPSIMD halves the rows (t1 = top + bottom),
            # DVE strided-reduces t1 for the xw partial. xh alternates:
            # even chunks reduce directly on DVE; odd chunks let GPSIMD
            # halve along w first, then DVE reduces the half. This splits
            # the pooling streams across both engines.
            HC2 = LOAD_HC // 2
            for b in range(NBLK):
                for j in range(n_load):
                    h0 = j * LOAD_HC
                    sl = slice(h0 * W, (h0 + LOAD_HC) * W)
                    nc.sync.dma_start(
                        out=xt[b][:, sl],
                        in_=x[b * P:(b + 1) * P, h0:h0 + LOAD_HC, :],
                    )
                    xv = xt[b][:, sl].rearrange("p (h w) -> p h w", h=LOAD_HC)
                    # xw: GPSIMD row-halving then DVE strided reduce over 14
                    t1 = wpool.tile([P, HC2 * W], F32, tag="t1")
                    nc.gpsimd.tensor_tensor(
                        t1[:, :],
                        xt[b][:, h0 * W:(h0 + HC2) * W],
                        xt[b][:, (h0 + HC2) * W:(h0 + LOAD_HC) * W],
                        OP.add,
                    )
                    nc.vector.tensor_reduce(
                        xw4[b][:, j * W:(j + 1) * W],
                        t1[:, :].rearrange("p (h w) -> p w h", h=HC2),
                        AX.X,
                        OP.add,
                    )
                    if j % 2 == 0:
                        # xh directly on DVE
                        nc.vector.tensor_reduce(
                            xh[b][:, h0:h0 + LOAD_HC], xv, AX.X, OP.add
                        )
                    else:
                        # GPSIMD halves along w, DVE reduces the half
                        u1 = wpool.tile([P, LOAD_HC * (W // 2)], F32, tag="u1")
                        u3 = u1[:, :].rearrange(
                            "p (h w) -> p h w", h=LOAD_HC
                        )
                        nc.gpsimd.tensor_tensor(
                            u3, xv[:, :, 0:W // 2], xv[:, :, W // 2:W], OP.add
                        )
                        nc.vector.tensor_reduce(
                            xh[b][:, h0:h0 + LOAD_HC], u3, AX.X, OP.add
                        )
                # fold the 4 partials
                nc.vector.tensor_reduce(
                    xw[b][:, :],
                    xw4[b][:, :].rearrange("p (j w) -> p w j", j=4),
                    AX.X,
                    OP.add,
                )

            # ---- conv chain (tiny) ----
            ph = ppool.tile([MIP, H], F32, tag="ph")
            pw = ppool.tile([MIP, W], F32, tag="pw")
            for b in range(NBLK):
                nc.tensor.matmul(
                    ph[:, :], w1t[:, b * MIP:(b + 1) * MIP], xh[b][:, :],
                    start=(b == 0), stop=(b == NBLK - 1),
                )
            for b in range(NBLK):
                nc.tensor.matmul(
                    pw[:, :], w2t[:, b * MIP:(b + 1) * MIP], xw[b][:, :],
                    start=(b == 0), stop=(b == NBLK - 1),
                )
            # relu(+bias) on DVE (keeps ACT tables on Sigmoid/Relu only)
            nc.vector.tensor_scalar(
                dh[:, :], ph[:, :], bbt[:, 0:1], 0.0, OP.add, OP.max
            )
            nc.vector.tensor_scalar(
                dw[:, :], pw[:, :], bbt[:, 1:2], 0.0, OP.add, OP.max
            )

            for b in range(NBLK):
                p2h = ppool.tile([P, H], F32, tag="p2h")
                p2w = ppool.tile([P, W], F32, tag="p2w")
                nc.tensor.matmul(
                    p2h[:, :], uswt[:, b * P:(b + 1) * P], dh[:, :],
                    start=True, stop=True,
                )
                nc.tensor.matmul(
                    p2w[:, :], uswt[:, b * P:(b + 1) * P], dw[:, :],
                    start=True, stop=True,
                )
                # ah' = -sigmoid(p2h + usb)/6 : sigmoid first, then scale
                nc.scalar.activation(
                    ahp[b][:, :], p2h[:, :], AF.Sigmoid, bias=usbt[:, b:b + 1]
                )
                nc.scalar.activation(
                    aw[b][:, :], p2w[:, :], AF.Sigmoid, bias=usbt[:, b:b + 1]
                )
            if not USE_FUSED:
                for b in range(NBLK):
                    nc.vector.tensor_scalar(
                        ahp[b][:, :], ahp[b][:, :], -1.0 / 6.0, None, OP.mult
                    )
            hsw = _hswish_op() if USE_FUSED else None

            # ---- phase 2: apply + store ----
            for b in range(NBLK):
                for j in range(n_apply):
                    h0 = j * APPLY_HC
                    sl = slice(h0 * W, (h0 + APPLY_HC) * W)
                    fd = APPLY_HC * W
                    sp = wpool.tile([P, fd], F32, tag="s")
                    f = wpool.tile([P, fd], F32, tag="f")

                    ah_v = (
                        ahp[b][:, h0:h0 + APPLY_HC]
                        .unsqueeze(2)
                        .broadcast_to([P, APPLY_HC, W])
                    )
                    aw_v = (
                        aw[b][:, :].unsqueeze(1).broadcast_to([P, APPLY_HC, W])
                    )
                    s3 = sp[:, :].rearrange("p (h w) -> p h w", h=APPLY_HC)
                    # ~60% of the s-passes go to GPSIMD to unload DVE
                    ci = b * n_apply + j
                    if ci % 8 in (0, 3, 6):
                        nc.vector.tensor_tensor(s3, ah_v, aw_v, OP.mult)
                    else:
                        nc.gpsimd.tensor_tensor(s3, ah_v, aw_v, OP.mult)
                    if USE_FUSED:
                        nc.vector._custom_dve(
                            hsw, out=f[:, :], in0=sp[:, :], in1=xt[b][:, sl],
                            s0=1.0, s1=1.0 / 6.0, imm2=0.5,
                        )
                    else:
                        op_ = wpool.tile([P, fd], F32, tag="o")
                        r1 = wpool.tile([P, fd], F32, tag="r1")
                        r2 = wpool.tile([P, fd], F32, tag="r2")
                        nc.vector.scalar_tensor_tensor(
                            op_[:, :], sp[:, :], -1.0 / 6.0, xt[b][:, sl],
                            OP.add, OP.mult,
                        )
                        nc.scalar.activation(
                            r1[:, :], op_[:, :], AF.Relu,
                            bias=cstt[:, 0:1], scale=-1.0,
                        )
                        nc.scalar.activation(
                            r2[:, :], r1[:, :], AF.Relu,
                            bias=cstt[:, 1:2], scale=-6.0,
                        )
                        nc.vector.scalar_tensor_tensor(
                            f[:, :], r2[:, :], -6.0, op_[:, :], OP.add, OP.mult
                        )
                    nc.sync.dma_start(
                        out=out[b * P:(b + 1) * P, h0:h0 + APPLY_HC, :],
                        in_=f[:, :].rearrange("p (h w) -> p h w", h=APPLY_HC),
                    )
    nc.compile()
    return nc


def _fold_params(bn1_gamma, bn1_beta, bn1_mean, bn1_var,
                 bn2_gamma, bn2_beta, bn2_mean, bn2_var,
                 ds_w, ds_b, us_w, us_b):
    s1 = bn1_gamma / np.sqrt(bn1_var + EPS)
    t1 = bn1_beta - bn1_mean * s1
    s2 = bn2_gamma / np.sqrt(bn2_var + EPS)
    t2 = bn2_beta - bn2_mean * s2
    # d = relu(ds_w @ (sum/W * s + t) + ds_b) = relu(W_eff @ sum + b_eff)
    w1e = (ds_w * s1[None, :] / W).astype(np.float32)      # [MIP, C]
    w2e = (ds_w * s2[None, :] / H).astype(np.float32)
    b1e = (ds_w @ t1 + ds_b).astype(np.float32)            # [MIP]
    b2e = (ds_w @ t2 + ds_b).astype(np.float32)
    # lhsT layouts
    w1_l = w1e.T.reshape(NBLK, P, MIP)                     # [blk, c, o]
    w2_l = w2e.T.reshape(NBLK, P, MIP)
    usw_l = us_w.T.astype(np.float32)                      # [MIP, C]
    usb_l = us_b.reshape(NBLK, P)
    bb_l = np.stack([b1e, b2e], axis=1)                    # [MIP, 2]

    pp = np.zeros((P, PPK), np.float32)
    pp[:, 0:MIP] = w1_l[0]
    pp[:, MIP:2 * MIP] = w1_l[1]
    pp[:, 16:16 + MIP] = w2_l[0]
    pp[:, 16 + MIP:32] = w2_l[1]
    pp[0:MIP, 32:32 + C] = usw_l
    pp[:, 288] = usb_l[0]
    pp[:, 289] = usb_l[1]
    pp[0:MIP, 290:292] = bb_l
    pp[:, 292] = 0.5
    pp[:, 293] = 6.0
    return np.ascontiguousarray(pp)


def kernel(**inputs) -> np.ndarray:
    x = np.asarray(inputs["x"], dtype=np.float32)
    pp = _fold_params(
        np.asarray(inputs["bn1_gamma"], np.float32),
        np.asarray(inputs["bn1_beta"], np.float32),
        np.asarray(inputs["bn1_mean"], np.float32),
        np.asarray(inputs["bn1_var"], np.float32),
        np.asarray(inputs["bn2_gamma"], np.float32),
        np.asarray(inputs["bn2_beta"], np.float32),
        np.asarray(inputs["bn2_mean"], np.float32),
        np.asarray(inputs["bn2_var"], np.float32),
        np.asarray(inputs["ds_w"], np.float32),
        np.asarray(inputs["ds_b"], np.float32),
        np.asarray(inputs["us_w"], np.float32),
        np.asarray(inputs["us_b"], np.float32),
    )
    if "nc" not in _CACHE:
        _CACHE["nc"] = _build_program()
    nc = _CACHE["nc"]

    in_maps = [
        {"x": np.ascontiguousarray(x[i]), "pp": pp} for i in range(N)
    ]
    res = run_bass_kernel_spmd(nc, in_maps, core_ids=list(range(N)))
    return np.stack([r["out"] for r in res.results]).astype(np.float32)


if __name__ == "__main__":
    rng = np.random.default_rng(0)
    demo = {
        "x": rng.standard_normal((N, C, H, W), dtype=np.float32),
        "bn1_gamma": rng.random(C, dtype=np.float32),
        "bn1_beta": rng.standard_normal(C).astype(np.float32) * 0.1,
        "bn1_mean": rng.standard_normal(C).astype(np.float32) * 0.1,
        "bn1_var": rng.random(C, dtype=np.float32) + 0.5,
        "bn2_gamma": rng.random(C, dtype=np.float32),
        "bn2_beta": rng.standard_normal(C).astype(np.float32) * 0.1,
        "bn2_mean": rng.standard_normal(C).astype(np.float32) * 0.1,
        "bn2_var": rng.random(C, dtype=np.float32) + 0.5,
        "ds_w": (rng.standard_normal((MIP, C)) / np.sqrt(C)).astype(np.float32),
        "ds_b": (rng.standard_normal(MIP) * 0.01).astype(np.float32),
        "us_w": (rng.standard_normal((C, MIP)) / np.sqrt(MIP)).astype(np.float32),
        "us_b": (rng.standard_normal(C) * 0.01).astype(np.float32),
    }
    o = kernel(**demo)
    print("kernel output", o.shape, o.dtype)


# revision 32
# speedup vs baseline: 1.1818x; 1.0506x over previous
"""CoordAtt (coordinate attention) Trainium2 Bass kernel.

Data-parallel over 8 NeuronCores: one batch sample per core.
Per-core layout: channels on partitions (2 blocks of 128), [H*W] on free dim.

Math (per sample):
  xh[c,h] = sum_w x[c,h,w];  xw[c,w] = sum_h x[c,h,w]          (pool, DVE)
  d_h = relu(W1eff @ xh + b1eff)   (BN + 1/W + conv folded on host, PE)
  d_w = relu(W2eff @ xw + b2eff)
  ah  = sigmoid(us_w @ d_h + us_b) ; aw = sigmoid(us_w @ d_w + us_b)
  o   = x*(1 + ah*aw);  out = hswish(o) = o*clip(o/6+1/2, 0, 1)

h-swish is computed exactly via:
  ah' = -ah/6 ; s' = ah'*aw ; o' = (s' - 1/6)*x          ( = -o/6 )
  r1 = relu(-o' + 1/2) ; r2 = relu(-6*r1 + 6) ; out = (r2 - 6)*o'
"""

import sys

if "/opt/trn_rl_repo" not in sys.path:
    sys.path.insert(0, "/opt/trn_rl_repo")

import numpy as np

import concourse.bass as bass
import concourse.mybir as mybir
from concourse.bass_utils import run_bass_kernel_spmd
from concourse.tile import TileContext

F32 = mybir.dt.float32
AF = mybir.ActivationFunctionType
OP = mybir.AluOpType
AX = mybir.AxisListType

N, C, H, W = 8, 256, 112, 112
HW = H * W
NBLK = 2  # channel blocks of 128
P = 128
MIP = 8
EPS = 1e-5

# chunk sizes (rows of h)
LOAD_HC = 28   # 4 load chunks per block
APPLY_HC = 28  # 4 apply chunks per block

_CACHE: dict = {}
PPK = 294  # packed param tile width
import os as _os

USE_FUSED = _os.environ.get("COORDATT_FUSED", "1") == "1"


def _hswish_op():
    """Register (once) a custom DVE op computing the whole apply tail:
    out = o * clip(o*s1 + imm2, 0, s0), o = (in0 + s0) * in1
    with s0=1, s1=1/6, imm2=1/2 this is hswish(x*(1+ah*aw)) given
    in0 = ah*aw, in1 = x.
    """
    from concourse import dve_ops as _dvo
    from concourse.dve_spec import Spec, Src0, Src1, C0, C1, C2, relu, minn

    name = "HSWISH_APPLY_ANT"
    for op in _dvo.OPS:
        if op.name == name:
            return op
    o = (Src0 + C0) * Src1

    def _ref(in0, in1, s0, s1, imm2):
        oo = ((in0.astype(np.float32) + s0) * in1).astype(np.float32)
        return (oo * np.clip(oo * s1 + imm2, 0.0, s0)).astype(np.float32)

    spec = Spec(body=o * relu(minn(o * C1 + C2, C0)), reference=_ref)
    op = _dvo.DveOp(
        name, spec, subdim=False,
        uops_sha={"v3": "4af51ed3c9ed553f", "v4": "b44642dd784bda88"},
    )
    _dvo.OPS.append(op)
    _dvo.CUSTOM_DVE_SPECS[name] = spec
    _dvo._SUB_OPCODE_FOR_NAME[name] = (
        _dvo._CUSTOM_DVE_ROW_BASE + len(_dvo.OPS) - 1
    )
    return op


def _build_program() -> bass.Bass:
    from concourse.bacc import Bacc

    nc = Bacc()

    x = nc.declare_dram_parameter("x", [C, H, W], F32, isOutput=False)
    pp = nc.declare_dram_parameter("pp", [P, PPK], F32, isOutput=False)
    out = nc.declare_dram_parameter("out", [C, H, W], F32, isOutput=True)

    n_load = H // LOAD_HC
    n_apply = H // APPLY_HC

    with TileContext(nc) as tc:
        with (
            tc.tile_pool(name="xres", bufs=1) as xpool,
            tc.tile_pool(name="small", bufs=1) as spool,
            tc.tile_pool(name="work", bufs=3) as wpool,
            tc.tile_pool(name="psum", bufs=2, space="PSUM") as ppool,
        ):
            # ---- resident x tiles + param tiles ----
            xt = [xpool.tile([P, HW], F32, tag=f"x{b}", name=f"x{b}") for b in range(NBLK)]
            pps = spool.tile([P, PPK], F32, tag="pps")
            ppt = spool.tile([P, PPK], F32, tag="pp")
            # packed param views (see _pack_params)
            w1t = ppt[:, 0:16]
            w2t = ppt[:, 16:32]
            uswt = ppt[0:MIP, 32:32 + C]
            usbt = ppt[:, 288:290]
            bbt = ppt[0:MIP, 290:292]
            cstt = ppt[:, 292:294]
            xh = [spool.tile([P, H], F32, tag=f"xh{b}", name=f"xh{b}") for b in range(NBLK)]
            xw4 = [spool.tile([P, 4 * W], F32, tag=f"xw4{b}", name=f"xw4{b}") for b in range(NBLK)]
            xw = [spool.tile([P, W], F32, tag=f"xw{b}", name=f"xw{b}") for b in range(NBLK)]
            dh = spool.tile([MIP, H], F32, tag="dh")
            dw = spool.tile([MIP, W], F32, tag="dw")
            ahp = [spool.tile([P, H], F32, tag=f"ahp{b}", name=f"ahp{b}") for b in range(NBLK)]
            aw = [spool.tile([P, W], F32, tag=f"aw{b}", name=f"aw{b}") for b in range(NBLK)]

            # single packed param load; bounce through DVE so downstream
            # matmuls depend on one DVE sem instead of a DMA-queue sem
            # (the Matmult LW struct carries very few sync waits).
            nc.sync.dma_start(out=pps[:, :], in_=pp[:, :])
            nc.vector.tensor_copy(ppt[:, :], pps[:, :])
            # ACT warmup: make the scalar engine observe the DVE sem now so
            # the first sigmoid later needs only the PE-sem wait (ISA allows
            # very few sync waits per instruction). The dummy sigmoid also
            # preloads the ACT table outside the critical path.
            warm = spool.tile([P, 1], F32, tag="warm")
            nc.scalar.copy(warm[:, :], cstt[:, 0:1])
            nc.scalar.activation(warm[:, :], cstt[:, 0:1], AF.Sigmoid)

            # ---- phase 1: load x, pooled sums ----
            # Per 28-row chunk: GPSIMD halves the rows (t1 = top + bottom),
            # DVE strided-reduces t1 for the xw partial. xh alternates:
            # even chunks reduce directly on DVE; odd chunks let GPSIMD
            # halve along w first, then DVE reduces the half. This splits
            # the pooling streams across both engines.
            HC2 = LOAD_HC // 2
            for b in range(NBLK):
                for j in range(n_load):
                    h0 = j * LOAD_HC
                    sl = slice(h0 * W, (h0 + LOAD_HC) * W)
                    nc.sync.dma_start(
                        out=xt[b][:, sl],
                        in_=x[b * P:(b + 1) * P, h0:h0 + LOAD_HC, :],
                    )
                    xv = xt[b][:, sl].rearrange("p (h w) -> p h w", h=LOAD_HC)
                    # xw: GPSIMD row-halving then DVE strided reduce over 14
                    t1 = wpool.tile([P, HC2 * W], F32, tag="t1", bufs=2)
                    nc.gpsimd.tensor_tensor(
                        t1[:, :],
                        xt[b][:, h0 * W:(h0 + HC2) * W],
                        xt[b][:, (h0 + HC2) * W:(h0 + LOAD_HC) * W],
                        OP.add,
                    )
                    nc.vector.tensor_reduce(
                        xw4[b][:, j * W:(j + 1) * W],
                        t1[:, :].rearrange("p (h w) -> p w h", h=HC2),
                        AX.X,
                        OP.add,
                    )
                    if j % 2 == 0:
                        # xh directly on DVE
                        nc.vector.tensor_reduce(
                            xh[b][:, h0:h0 + LOAD_HC], xv, AX.X, OP.add
                        )
                    else:
                        # GPSIMD halves along w, DVE reduces the half
                        u1 = wpool.tile(
                            [P, LOAD_HC * (W // 2)], F32, tag="u1", bufs=2
                        )
                        u3 = u1[:, :].rearrange(
                            "p (h w) -> p h w", h=LOAD_HC
                        )
                        nc.gpsimd.tensor_tensor(
                            u3, xv[:, :, 0:W // 2], xv[:, :, W // 2:W], OP.add
                        )
                        nc.vector.tensor_reduce(
                            xh[b][:, h0:h0 + LOAD_HC], u3, AX.X, OP.add
                        )
                # fold the 4 partials
                nc.vector.tensor_reduce(
                    xw[b][:, :],
                    xw4[b][:, :].rearrange("p (j w) -> p w j", j=4),
                    AX.X,
                    OP.add,
                )

            # ---- conv chain (tiny) ----
            ph = ppool.tile([MIP, H], F32, tag="ph")
            pw = ppool.tile([MIP, W], F32, tag="pw")
            for b in range(NBLK):
                nc.tensor.matmul(
                    ph[:, :], w1t[:, b * MIP:(b + 1) * MIP], xh[b][:, :],
                    start=(b == 0), stop=(b == NBLK - 1),
                )
            for b in range(NBLK):
                nc.tensor.matmul(
                    pw[:, :], w2t[:, b * MIP:(b + 1) * MIP], xw[b][:, :],
                    start=(b == 0), stop=(b == NBLK - 1),
                )
            # relu(+bias) on DVE (keeps ACT tables on Sigmoid/Relu only)
            nc.vector.tensor_scalar(
                dh[:, :], ph[:, :], bbt[:, 0:1], 0.0, OP.add, OP.max
            )
            nc.vector.tensor_scalar(
                dw[:, :], pw[:, :], bbt[:, 1:2], 0.0, OP.add, OP.max
            )

            for b in range(NBLK):
                p2h = ppool.tile([P, H], F32, tag="p2h")
                p2w = ppool.tile([P, W], F32, tag="p2w")
                nc.tensor.matmul(
                    p2h[:, :], uswt[:, b * P:(b + 1) * P], dh[:, :],
                    start=True, stop=True,
                )
                nc.tensor.matmul(
                    p2w[:, :], uswt[:, b * P:(b + 1) * P], dw[:, :],
                    start=True, stop=True,
                )
                # ah' = -sigmoid(p2h + usb)/6 : sigmoid first, then scale
                nc.scalar.activation(
                    ahp[b][:, :], p2h[:, :], AF.Sigmoid, bias=usbt[:, b:b + 1]
                )
                nc.scalar.activation(
                    aw[b][:, :], p2w[:, :], AF.Sigmoid, bias=usbt[:, b:b + 1]
                )
            if not USE_FUSED:
                for b in range(NBLK):
                    nc.vector.tensor_scalar(
                        ahp[b][:, :], ahp[b][:, :], -1.0 / 6.0, None, OP.mult
                    )
            hsw = _hswish_op() if USE_FUSED else None

            # ---- phase 2: apply + store ----
            for b in range(NBLK):
                for j in range(n_apply):
                    h0 = j * APPLY_HC
                    sl = slice(h0 * W, (h0 + APPLY_HC) * W)
                    fd = APPLY_HC * W
                    sp = wpool.tile([P, fd], F32, tag="s", bufs=2)
                    f = wpool.tile([P, fd], F32, tag="f", bufs=2)

                    ah_v = (
                        ahp[b][:, h0:h0 + APPLY_HC]
                        .unsqueeze(2)
                        .broadcast_to([P, APPLY_HC, W])
                    )
                    aw_v = (
                        aw[b][:, :].unsqueeze(1).broadcast_to([P, APPLY_HC, W])
                    )
                    s3 = sp[:, :].rearrange("p (h w) -> p h w", h=APPLY_HC)
                    # all-DVE apply: GPSIMD concurrency taxes DVE 2-port ops
                    # (shared SBUF port), so phase 2 stays off GPSIMD
                    nc.vector.tensor_tensor(s3, ah_v, aw_v, OP.mult)
                    if USE_FUSED:
                        nc.vector._custom_dve(
                            hsw, out=f[:, :], in0=sp[:, :], in1=xt[b][:, sl],
                            s0=1.0, s1=1.0 / 6.0, imm2=0.5,
                        )
                    else:
                        op_ = wpool.tile([P, fd], F32, tag="o")
                        r1 = wpool.tile([P, fd], F32, tag="r1")
                        r2 = wpool.tile([P, fd], F32, tag="r2")
                        nc.vector.scalar_tensor_tensor(
                            op_[:, :], sp[:, :], -1.0 / 6.0, xt[b][:, sl],
                            OP.add, OP.mult,
                        )
                        nc.scalar.activation(
                            r1[:, :], op_[:, :], AF.Relu,
                            bias=cstt[:, 0:1], scale=-1.0,
                        )
                        nc.scalar.activation(
                            r2[:, :], r1[:, :], AF.Relu,
                            bias=cstt[:, 1:2], scale=-6.0,
                        )
                        nc.vector.scalar_tensor_tensor(
                            f[:, :], r2[:, :], -6.0, op_[:, :], OP.add, OP.mult
                        )
                    nc.sync.dma_start(
                        out=out[b * P:(b + 1) * P, h0:h0 + APPLY_HC, :],
                        in_=f[:, :].rearrange("p (h w) -> p h w", h=APPLY_HC),
                    )
    nc.compile()
    return nc


def _fold_params(bn1_gamma, bn1_beta, bn1_mean, bn1_var,
                 bn2_gamma, bn2_beta, bn2_mean, bn2_var,
                 ds_w, ds_b, us_w, us_b):
    s1 = bn1_gamma / np.sqrt(bn1_var + EPS)
    t1 = bn1_beta - bn1_mean * s1
    s2 = bn2_gamma / np.sqrt(bn2_var + EPS)
    t2 = bn2_beta - bn2_mean * s2
    # d = relu(ds_w @ (sum/W * s + t) + ds_b) = relu(W_eff @ sum + b_eff)
    w1e = (ds_w * s1[None, :] / W).astype(np.float32)      # [MIP, C]
    w2e = (ds_w * s2[None, :] / H).astype(np.float32)
    b1e = (ds_w @ t1 + ds_b).astype(np.float32)            # [MIP]
    b2e = (ds_w @ t2 + ds_b).astype(np.float32)
    # lhsT layouts
    w1_l = w1e.T.reshape(NBLK, P, MIP)                     # [blk, c, o]
    w2_l = w2e.T.reshape(NBLK, P, MIP)
    usw_l = us_w.T.astype(np.float32)                      # [MIP, C]
    usb_l = us_b.reshape(NBLK, P)
    bb_l = np.stack([b1e, b2e], axis=1)                    # [MIP, 2]

    pp = np.zeros((P, PPK), np.float32)
    pp[:, 0:MIP] = w1_l[0]
    pp[:, MIP:2 * MIP] = w1_l[1]
    pp[:, 16:16 + MIP] = w2_l[0]
    pp[:, 16 + MIP:32] = w2_l[1]
    pp[0:MIP, 32:32 + C] = usw_l
    pp[:, 288] = usb_l[0]
    pp[:, 289] = usb_l[1]
    pp[0:MIP, 290:292] = bb_l
    pp[:, 292] = 0.5
    pp[:, 293] = 6.0
    return np.ascontiguousarray(pp)


def kernel(**inputs) -> np.ndarray:
    x = np.asarray(inputs["x"], dtype=np.float32)
    pp = _fold_params(
        np.asarray(inputs["bn1_gamma"], np.float32),
        np.asarray(inputs["bn1_beta"], np.float32),
        np.asarray(inputs["bn1_mean"], np.float32),
        np.asarray(inputs["bn1_var"], np.float32),
        np.asarray(inputs["bn2_gamma"], np.float32),
        np.asarray(inputs["bn2_beta"], np.float32),
        np.asarray(inputs["bn2_mean"], np.float32),
        np.asarray(inputs["bn2_var"], np.float32),
        np.asarray(inputs["ds_w"], np.float32),
        np.asarray(inputs["ds_b"], np.float32),
        np.asarray(inputs["us_w"], np.float32),
        np.asarray(inputs["us_b"], np.float32),
    )
    if "nc" not in _CACHE:
        _CACHE["nc"] = _build_program()
    nc = _CACHE["nc"]

    in_maps = [
        {"x": np.ascontiguousarray(x[i]), "pp": pp} for i in range(N)
    ]
    res = run_bass_kernel_spmd(nc, in_maps, core_ids=list(range(N)))
    return np.stack([r["out"] for r in res.results]).astype(np.float32)


if __name__ == "__main__":
    rng = np.random.default_rng(0)
    demo = {
        "x": rng.standard_normal((N, C, H, W), dtype=np.float32),
        "bn1_gamma": rng.random(C, dtype=np.float32),
        "bn1_beta": rng.standard_normal(C).astype(np.float32) * 0.1,
        "bn1_mean": rng.standard_normal(C).astype(np.float32) * 0.1,
        "bn1_var": rng.random(C, dtype=np.float32) + 0.5,
        "bn2_gamma": rng.random(C, dtype=np.float32),
        "bn2_beta": rng.standard_normal(C).astype(np.float32) * 0.1,
        "bn2_mean": rng.standard_normal(C).astype(np.float32) * 0.1,
        "bn2_var": rng.random(C, dtype=np.float32) + 0.5,
        "ds_w": (rng.standard_normal((MIP, C)) / np.sqrt(C)).astype(np.float32),
        "ds_b": (rng.standard_normal(MIP) * 0.01).astype(np.float32),
        "us_w": (rng.standard_normal((C, MIP)) / np.sqrt(MIP)).astype(np.float32),
        "us_b": (rng.standard_normal(C) * 0.01).astype(np.float32),
    }
    o = kernel(**demo)
    print("kernel output", o.shape, o.dtype)
